# revision 15
# baseline (speedup 1.0000x reference)
"""Trainium2 Bass kernel for a 2-layer LSTM (B=256, T=512, I=64, H=256) + linear head.

Strategy (hardcoded, self-contained):
  - Data-parallel over batch across 8 NeuronCores (32 batch elems per core).
  - MERGED PHASES: phase p advances layer 0 by one step (t=p) AND layer 1 by
    one step (t=p-2) through ONE double-wide instruction set, halving the
    per-step fixed instruction overheads (ACT ~293ns/instr, DVE ~60-110ns,
    semaphore hops ~100ns) that dominate the serial recurrence chain:
      gate PSUM tile g01 [128=(hblk4, b32), 512=(layer2, gate4, hh2, hl32)]
      one sigmoid over (i,f,g) of both layers, one over (o), one tanh(c),
      one m1/cf/add/h DVE set on [128,128] tiles, ONE 32x32-block transpose
      producing both layers' next stationaries hT01.
  - Gate order (i, f, g, o) per layer so the chain-critical sigmoid(i,f,g)
    is a single 2-block strided AP.
  - tanh(g) is folded into sigmoid via tanh(x) = 2*sigmoid(2x)-1 (g-gate
    weight columns x2 host-side); cell state kept as C' = c/2 so the update
    is a plain add: C' = sig(f)*C' + (sig(2g)-0.5)*sig(i), and
    tanh(c) = tanh(2*C') via the ACT engine's free input scale.
  - All matmul operands bf16 (PSUM fp32); col-group-packed matmuls
    (tile_position=(0,32m)) share small transposed-state stationaries.
  - Layer 1 lags by 2 phases so its input projection (hT0 from 2 phases
    back) is off the critical chain; both layers' recurrent passes gate on
    the single previous-phase transpose.
  - Input projection x@Wih.T and biases ride the PSUM accumulation
    (augmented ones-row / ones-stationary trick).
  - The two output linear layers are folded host-side into one [256,4]
    matmul + bias.
"""

import numpy as np

B, T, I, H, O = 256, 512, 64, 256, 4
NCORES = 8
BS = B // NCORES  # 32

# gate order: reference (i, f, g, o) kept as-is; cols per layer-block:
# 0:64=i, 64:128=f, 128:192=g(x2 prescale), 192:256=o.
GATE_PERM = [0, 1, 2, 3]

# weight blob column offsets (bf16 elements, [128, WB_COLS])
OFF_W0 = 0        # Whh0 perm  [128, 2*1024]
OFF_W1 = 2048     # Whh1 perm  [128, 2*1024]
OFF_WX1 = 4096    # Wih1 perm  [128, 2*1024]
OFF_WX0 = 6144    # Wih0 perm + bias row, rows 0:65, [65, 1024]
OFF_B1 = 7168     # bias1 row, row 0, [1, 1024]
OFF_WF = 8192     # folded head weight [128, 2*4]
OFF_BF = 8200     # folded head bias, row 0, [1, 4]
OFF_XT = 8224     # x transposed + ones row, rows 0:65, [65, t_steps*32]
def _wb_cols(t_steps):
    return OFF_XT + t_steps * BS

_CACHED = {}


def _perm_cols(Wt):
    """Permute gate columns of [K, 1024] (col j = gate_orig*256 + h) into
    col = m*256 + gate_new*64 + hh*32 + hl, where h = hh*128 + m*32 + hl."""
    K = Wt.shape[0]
    W = Wt.reshape(K, 4, 256)[:, GATE_PERM, :]      # [K, gate, h]
    W = W.reshape(K, 4, 2, 4, 32)                    # [K, gate, hh, m, hl]
    W = W.transpose(0, 3, 1, 2, 4)                   # [K, m, gate, hh, hl]
    return np.ascontiguousarray(W.reshape(K, 1024))


def _build_bass(t_steps=T):
    import concourse.mybir as mybir
    import concourse.tile as tile
    from concourse import bacc
    from contextlib import ExitStack

    f32 = mybir.dt.float32
    f16 = mybir.dt.float16
    bf16 = mybir.dt.bfloat16
    AF = mybir.ActivationFunctionType
    ALU = mybir.AluOpType

    nc = bacc.Bacc("TRN2", target_bir_lowering=False)

    wb_cols = _wb_cols(t_steps)
    wb_d = nc.dram_tensor("wb", (128, wb_cols), bf16, kind="ExternalInput")
    y_d = nc.dram_tensor("y", (BS, O), f32, kind="ExternalOutput")

    with tile.TileContext(nc) as tc, ExitStack() as ctx:
        const = ctx.enter_context(tc.tile_pool(name="const", bufs=1))
        cst = ctx.enter_context(tc.tile_pool(name="cst", bufs=3))
        work = ctx.enter_context(tc.tile_pool(name="work", bufs=4))
        hts = ctx.enter_context(tc.tile_pool(name="hts", bufs=5))
        psum = ctx.enter_context(tc.tile_pool(name="psum", bufs=3, space="PSUM"))

        wb = const.tile([128, wb_cols], bf16)
        nc.sync.dma_start(wb[:], wb_d[:])

        def xt_ap(t):
            return wb[0:65, OFF_XT + BS * t : OFF_XT + BS * t + BS]

        def w0_ap(kc, m):
            return wb[:, OFF_W0 + 1024 * kc + 256 * m : OFF_W0 + 1024 * kc + 256 * m + 256]

        def w1_ap(kc, m):
            return wb[:, OFF_W1 + 1024 * kc + 256 * m : OFF_W1 + 1024 * kc + 256 * m + 256]

        def wx1_ap(kc, m):
            return wb[:, OFF_WX1 + 1024 * kc + 256 * m : OFF_WX1 + 1024 * kc + 256 * m + 256]

        def wx0_ap(m):
            return wb[0:65, OFF_WX0 + 256 * m : OFF_WX0 + 256 * m + 256]

        def b1_ap(m):
            return wb[0:1, OFF_B1 + 256 * m : OFF_B1 + 256 * m + 256]

        # zero-initialized state: combined hT01 [128, 128] (cols 0:64 = L0's
        # hT, 64:128 = L1's) and combined C' [128, 128] (cols 0:64 L0 cell,
        # 64:128 L1 cell).
        hT_z = const.tile([128, 128], bf16)
        nc.vector.memset(hT_z[:], 0.0)
        c01 = const.tile([128, 128], f16)
        nc.vector.memset(c01[:], 0.0)
        ones_t = const.tile([1, BS], bf16)
        nc.vector.memset(ones_t[:], 1.0)
        ones_ap = ones_t[:]

        def l2(ap):
            # [128, 512] -> [128, 2(layer), 256]
            return ap.rearrange("p (l c) -> p l c", l=2)

        def l2h(ap):
            # [128, 128] -> [128, 2(layer), 64]
            return ap.rearrange("p (l c) -> p l c", l=2)

        hist = [hT_z, hT_z]  # [hT01(p-2), hT01(p-1)]
        for p in range(t_steps + 2):
            t0 = min(p, t_steps - 1)  # L0 step (garbage recompute at tail)
            prev2, prev = hist[-2], hist[-1]
            g0t = psum.tile([128, 256], f32, tag="ga")
            g1t = psum.tile([128, 256], f32, tag="gb")

            # ---- matmuls ----
            # L0 x-projection + bias (start), early
            for m in range(4):
                nc.tensor.matmul(
                    g0t[32 * m : 32 * m + 32, :], xt_ap(t0), wx0_ap(m),
                    start=True, stop=False, tile_position=(0, 32 * m), skip_group_check=True,
                )
            # L1 bias (start), early. Skipped for p<2 so the L1 half stays
            # exactly zero through the warmup phases (sig(0)-0.5 = 0).
            if p >= 2:
                for m in range(4):
                    nc.tensor.matmul(
                        g1t[32 * m : 32 * m + 32, :], ones_ap, b1_ap(m),
                        start=True, stop=False, tile_position=(0, 32 * m), skip_group_check=True,
                    )
            # L1 input projection from hT0(p-2) (off-chain: 2 phases old)
            for kc in range(2):
                for m in range(4):
                    nc.tensor.matmul(
                        g1t[32 * m : 32 * m + 32, :],
                        prev2[:, 32 * kc : 32 * kc + 32], wx1_ap(kc, m),
                        start=(p < 2 and kc == 0), stop=False,
                        tile_position=(0, 32 * m), skip_group_check=True,
                    )
            # L1 recurrence from hT1(p-3) (prev phase's transpose)
            for kc in range(2):
                for m in range(4):
                    nc.tensor.matmul(
                        g1t[32 * m : 32 * m + 32, :],
                        prev[:, 64 + 32 * kc : 64 + 32 * kc + 32], w1_ap(kc, m),
                        start=False, stop=(kc == 1),
                        tile_position=(0, 32 * m), skip_group_check=True,
                    )
            # L0 recurrence from hT0(p-1) (prev phase's transpose) -- last
            for kc in range(2):
                for m in range(4):
                    nc.tensor.matmul(
                        g0t[32 * m : 32 * m + 32, :],
                        prev[:, 32 * kc : 32 * kc + 32], w0_ap(kc, m),
                        start=False, stop=(kc == 1),
                        tile_position=(0, 32 * m), skip_group_check=True,
                    )

            # ---- elementwise (double-wide: both layers in one op set) ----
            sg = work.tile([128, 512], f16, tag="sg")
            sgl = l2(sg[:])
            # DBG: two sigmoids from split PSUM tiles
            nc.scalar.activation(sg[:, 0:256], g0t[:], AF.Sigmoid)
            nc.scalar.activation(sg[:, 256:512], g1t[:], AF.Sigmoid)
            # m1 = (sig(2g) - 0.5) * sig(i)   [DBG: per-layer 2D]
            m1 = work.tile([128, 128], f16, tag="m1")
            nc.vector.scalar_tensor_tensor(
                m1[:, 0:64], sg[:, 128:192], 0.5, sg[:, 0:64], ALU.subtract, ALU.mult)
            nc.vector.scalar_tensor_tensor(
                m1[:, 64:128], sg[:, 384:448], 0.5, sg[:, 256:320], ALU.subtract, ALU.mult)
            # cf = sig(f) * C'_prev
            cf = work.tile([128, 128], f16, tag="cf")
            nc.vector.tensor_mul(cf[:, 0:64], sg[:, 64:128], c01[:, 0:64])
            nc.vector.tensor_mul(cf[:, 64:128], sg[:, 320:384], c01[:, 64:128])
            # C' = m1 + cf
            c_new = cst.tile([128, 128], f16, tag="c")
            nc.vector.tensor_add(c_new[:], m1[:], cf[:])
            # tc = tanh(2*C') = tanh(c)
            sc = work.tile([128, 128], f16, tag="sc")
            nc.scalar.activation(sc[:], c_new[:], AF.Tanh, scale=2.0)
            # h = sig(o) * tanh(c)   [DBG: per-layer 2D]
            h = work.tile([128, 128], bf16, tag="h")
            nc.vector.tensor_mul(h[:, 0:64], sc[:, 0:64], sg[:, 192:256])
            nc.vector.tensor_mul(h[:, 64:128], sc[:, 64:128], sg[:, 448:512])
            # single transpose -> both layers' next stationaries
            hT = hts.tile([128, 128], bf16, tag="ht")
            nc.vector.transpose(hT[:], h[:])

            hist = [prev, hT]
            c01 = c_new

        # ---- head: y = hT1_final.T @ Wf + bf ----
        hT_f = hist[-1]
        yp = psum.tile([BS, O], f32, tag="yh", bufs=1)
        nc.tensor.matmul(yp[:], ones_ap, wb[0:1, OFF_BF : OFF_BF + O], start=True, stop=False)
        nc.tensor.matmul(yp[:], hT_f[:, 64:96], wb[:, OFF_WF : OFF_WF + O], start=False, stop=False)
        nc.tensor.matmul(yp[:], hT_f[:, 96:128], wb[:, OFF_WF + O : OFF_WF + 2 * O], start=False, stop=True)
        y_sb = work.tile([BS, O], f32, tag="y")
        nc.vector.tensor_copy(y_sb[:], yp[:])
        nc.sync.dma_start(y_d[:], y_sb[:])

    return nc


def _scaled(W, b, hin_scale):
    """Apply the sigmoid-folding scale to a weight [4H, K] and bias [4H] in
    ORIGINAL (i, f, g, o) gate order: g-gate rows x2 (sigmoid(2x) prescale)."""
    W = np.asarray(W, np.float64).copy()
    b = np.asarray(b, np.float64).copy() if b is not None else None
    W[2 * H : 3 * H] *= 2.0
    W *= hin_scale
    if b is not None:
        b[2 * H : 3 * H] *= 2.0
    return W, b


def _prep_inputs(x, Wih0, Whh0, bih0, bhh0, Wih1, Whh1, bih1, bhh1, W1, b1, W2, b2,
                 t_steps=T):
    x = np.asarray(x, dtype=np.float32)[:, :t_steps, :]
    wb = np.zeros((128, _wb_cols(t_steps)), np.float64)
    sWhh0, _ = _scaled(Whh0, None, 1.0)
    sWih0, sb0 = _scaled(Wih0, np.asarray(bih0, np.float64) + np.asarray(bhh0, np.float64), 1.0)
    sWhh1, _ = _scaled(Whh1, None, 1.0)
    sWih1, sb1 = _scaled(Wih1, np.asarray(bih1, np.float64) + np.asarray(bhh1, np.float64), 1.0)

    wb[:, OFF_W0 : OFF_W0 + 2048] = _perm_cols(
        sWhh0.T).reshape(2, 128, 1024).transpose(1, 0, 2).reshape(128, 2048)
    wb[:, OFF_W1 : OFF_W1 + 2048] = _perm_cols(
        sWhh1.T).reshape(2, 128, 1024).transpose(1, 0, 2).reshape(128, 2048)
    wb[:, OFF_WX1 : OFF_WX1 + 2048] = _perm_cols(
        sWih1.T).reshape(2, 128, 1024).transpose(1, 0, 2).reshape(128, 2048)
    wb[0:64, OFF_WX0 : OFF_WX0 + 1024] = _perm_cols(sWih0.T)
    wb[64, OFF_WX0 : OFF_WX0 + 1024] = _perm_cols(sb0[None, :])[0]
    wb[0, OFF_B1 : OFF_B1 + 1024] = _perm_cols(sb1[None, :])[0]
    # head folded: y = h2*(W1.T@W2.T) + (b1@W2.T + b2)
    Wf = np.asarray(W1, np.float64).T @ np.asarray(W2, np.float64).T
    wb[:, OFF_WF : OFF_WF + 2 * O] = Wf.reshape(2, 128, O).transpose(1, 0, 2).reshape(128, 2 * O)
    wb[0, OFF_BF : OFF_BF + O] = (
        np.asarray(b1, np.float64) @ np.asarray(W2, np.float64).T + np.asarray(b2, np.float64))
    import ml_dtypes
    wb = wb.astype(ml_dtypes.bfloat16)

    in_maps = []
    for c in range(NCORES):
        xc = x[c * BS : (c + 1) * BS]                       # [BS, t, I]
        xt = xc.transpose(2, 1, 0).reshape(I, t_steps * BS) # [I, t*BS]
        wbc = wb.copy()
        wbc[0:64, OFF_XT:] = xt.astype(ml_dtypes.bfloat16)
        wbc[64, OFF_XT:] = 1.0
        in_maps.append(dict(wb=wbc))
    return in_maps


def run(t_steps=T, trace=False, **inputs):
    from concourse.bass_utils import run_bass_kernel_spmd

    key = t_steps
    if key not in _CACHED:
        nc_new = _build_bass(t_steps)
        # finalize BEFORE handing to the PJRT path: the bass_exec lowering
        # otherwise finalizes with the partition-id register preamble in a
        # state that miscompiles (walrus "Reg has not been allocated yet")
        nc_new.finalize()
        _CACHED[key] = nc_new
    nc = _CACHED[key]
    in_maps = _prep_inputs(**inputs, t_steps=t_steps)
    res = None
    for attempt in range(4):
        try:
            res = run_bass_kernel_spmd(nc, in_maps, core_ids=list(range(NCORES)),
                                       trace=trace)
            break
        except Exception as e:  # flaky parallel-birverifier race in neuronx-cc
            if attempt == 3:
                raise
            print(f"run attempt {attempt} failed ({type(e).__name__}); retrying")
    assert res is not None
    y = np.concatenate([r["y"] for r in res.results], axis=0)
    return y, res


def kernel(**inputs):
    y, _ = run(t_steps=T, trace=False, **inputs)
    return y
